# revision 10
# baseline (speedup 1.0000x reference)
"""BitLinear158 (LayerNorm -> int8 fake-quant -> ternary matmul -> LayerNorm)
on 8 Trainium2 NeuronCores, data-parallel over tokens.

Math notes (vs the fp32 reference):
  - Input LayerNorm's rstd cancels inside the activation quantizer:
        q = round(xn / (max|xn|/127)) = round((x-mu) * 127 / max|x-mu|)
    so the input-side sqrt/reciprocal of the variance is never needed.
  - q in [-127,127] and ternary weights {-1,0,1} are exact in bf16, and the
    PE accumulates in fp32, so the matmul integer arithmetic is exact.
  - The final LayerNorm is invariant to the per-token positive scale
    (x_scale), up to the eps term (~1e-5 relative), so x_quant*x_scale is
    never materialized; weight_scale is applied in fp32 after the matmul.
  - round-half-to-even is implemented with the fp32 magic-number trick:
    t = fma(v, c, 1.5*2^23); q = t - 1.5*2^23.
"""

from contextlib import ExitStack

import numpy as np
import ml_dtypes

N_CORES = 8
B, S, DIN, DOUT = 4, 4096, 2048, 2048
M_TOTAL = B * S
M_PER_CORE = M_TOTAL // N_CORES
P = 128
NBLK = M_PER_CORE // P          # token blocks per core
KT = DIN // P                   # contraction subtiles
NT = DOUT // 512                # psum bank tiles
EPS = 1e-5
MAGIC = float(np.float32(1.5 * 2 ** 23))

_CACHE = {}


def _build_nc(m_per_core=M_PER_CORE, repeats=1, loop_repeats=0):
    key = ("nc", m_per_core, repeats, loop_repeats)
    if key in _CACHE:
        return _CACHE[key]
    NBLK = m_per_core // P

    import concourse.bacc as bacc
    import concourse.tile as tile
    from concourse import mybir

    f32 = mybir.dt.float32
    bf16 = mybir.dt.bfloat16
    X = mybir.AxisListType.X
    Identity = mybir.ActivationFunctionType.Identity
    Sqrt = mybir.ActivationFunctionType.Sqrt

    nc = bacc.Bacc("TRN2", target_bir_lowering=False, num_devices=N_CORES,
                   name="bitlinear158")
    xs = nc.dram_tensor("xs", [m_per_core, DIN], f32, kind="ExternalInput")
    wt = nc.dram_tensor("wt", [DIN, DOUT], bf16, kind="ExternalInput")
    wsc = nc.dram_tensor("wsc", [1, DOUT], f32, kind="ExternalInput")
    out = nc.dram_tensor("out", [m_per_core, DOUT], f32, kind="ExternalOutput")

    with tile.TileContext(nc) as tc, ExitStack() as ctx:
        singles = ctx.enter_context(tc.tile_pool(name="singles", bufs=1))
        xp = ctx.enter_context(tc.tile_pool(name="xp", bufs=3))
        xcp = ctx.enter_context(tc.tile_pool(name="xcp", bufs=2))
        qp = ctx.enter_context(tc.tile_pool(name="qp", bufs=2))
        qtp = ctx.enter_context(tc.tile_pool(name="qtp", bufs=2))
        gp = ctx.enter_context(tc.tile_pool(name="gp", bufs=2))
        stp = ctx.enter_context(tc.tile_pool(name="stp", bufs=8))
        psp = ctx.enter_context(tc.tile_pool(name="psp", bufs=2, space="PSUM"))

        w_sb = singles.tile([P, KT, DOUT], bf16)
        nc.sync.dma_start(out=w_sb, in_=wt.rearrange("(kt p) n -> p kt n", p=P))
        wsc_sb = singles.tile([P, DOUT], f32)
        nc.sync.dma_start(out=wsc_sb, in_=wsc.ap().to_broadcast((P, DOUT)))
        eps_t = singles.tile([P, 1], f32)
        nc.vector.memset(eps_t, EPS)
        magic_t = singles.tile([P, 1], f32)
        nc.vector.memset(magic_t, MAGIC)

        loop_ctx = (tc.For_i(0, loop_repeats, 1) if loop_repeats
                    else ExitStack())
        with loop_ctx:
         for rep in range(repeats):
          for blk in range(NBLK):
            rows = slice(blk * P, (blk + 1) * P)

            x_t = xp.tile([P, DIN], f32)
            nc.sync.dma_start(out=x_t, in_=xs[rows, :])

            # ---- input LayerNorm + 8-bit absmax quant (rstd-free form) ----
            ssum = stp.tile([P, 1], f32)
            nc.vector.reduce_sum(out=ssum, in_=x_t, axis=X)
            negmu = stp.tile([P, 1], f32)
            nc.vector.tensor_scalar_mul(negmu, ssum, -1.0 / DIN)

            xc_t = xcp.tile([P, DIN], f32)          # x - mu
            nc.scalar.activation(out=xc_t, in_=x_t, func=Identity,
                                 bias=negmu, scale=1.0)

            amax = stp.tile([P, 1], f32)            # max |x - mu|
            nc.vector.tensor_reduce(out=amax, in_=xc_t, axis=X,
                                    op=mybir.AluOpType.max,
                                    apply_absolute_value=True)
            c127 = stp.tile([P, 1], f32)            # 127 / amax
            nc.vector.reciprocal(out=c127, in_=amax)
            nc.vector.tensor_scalar_mul(c127, c127, 127.0)

            # t = xc*c + MAGIC  (rounds to integer, RNE);  q = t - MAGIC
            nc.scalar.activation(out=xc_t, in_=xc_t, func=Identity,
                                 bias=magic_t, scale=c127)
            q_t = qp.tile([P, DIN], bf16)
            nc.vector.tensor_scalar(q_t, xc_t, MAGIC, None,
                                    op0=mybir.AluOpType.subtract)

            # ---- transpose q to contraction-major for the PE ----
            qT_t = qtp.tile([P, DIN], bf16)
            for kt in range(KT):
                cols = slice(kt * P, (kt + 1) * P)
                nc.sync.dma_start_transpose(out=qT_t[:, cols], in_=q_t[:, cols])

            # ---- exact integer matmul in bf16: psum = q @ ternary.T ----
            ps = psp.tile([P, DOUT], f32)
            for nt in range(NT):
                ncols = slice(nt * 512, (nt + 1) * 512)
                for kt in range(KT):
                    nc.tensor.matmul(ps[:, ncols],
                                     lhsT=qT_t[:, kt * P:(kt + 1) * P],
                                     rhs=w_sb[:, kt, ncols],
                                     start=(kt == 0), stop=(kt == KT - 1))

            # ---- g = psum * weight_scale; output LayerNorm stats ----
            g_t = gp.tile([P, DOUT], f32)
            nc.vector.tensor_mul(g_t, ps, wsc_sb)

            st2 = stp.tile([P, 4, 6], f32)
            for sg in range(4):
                nc.vector.bn_stats(out=st2[:, sg, :],
                                   in_=g_t[:, sg * 512:(sg + 1) * 512])
            mv2 = stp.tile([P, 2], f32)
            nc.vector.bn_aggr(out=mv2, in_=st2)

            rstd2 = stp.tile([P, 1], f32)
            nc.scalar.activation(out=rstd2, in_=mv2[:, 1:2], func=Sqrt,
                                 bias=eps_t, scale=1.0)
            nc.vector.reciprocal(out=rstd2, in_=rstd2)
            nb2 = stp.tile([P, 1], f32)
            nc.vector.tensor_scalar_mul(nb2, mv2[:, 0:1], -1.0)
            nc.vector.tensor_mul(nb2, nb2, rstd2)

            nc.scalar.activation(out=g_t, in_=g_t, func=Identity,
                                 bias=nb2, scale=rstd2)
            nc.sync.dma_start(out=out[rows, :], in_=g_t)

    nc.compile()
    _CACHE[key] = nc
    return nc


def _prep_in_maps(x, weight_ternary, weight_scale):
    xs = np.ascontiguousarray(
        np.asarray(x, dtype=np.float32).reshape(M_TOTAL, DIN))
    wt = np.ascontiguousarray(
        np.asarray(weight_ternary).astype(np.float32).T.astype(
            ml_dtypes.bfloat16))
    wsc = np.ascontiguousarray(
        np.asarray(weight_scale, dtype=np.float32).reshape(1, DOUT))
    return [
        {"xs": np.ascontiguousarray(xs[c * M_PER_CORE:(c + 1) * M_PER_CORE]),
         "wt": wt, "wsc": wsc}
        for c in range(N_CORES)
    ]


def run(x, weight_ternary, weight_scale, trace=False):
    from concourse.bass_utils import run_bass_kernel_spmd
    nc = _build_nc()
    in_maps = _prep_in_maps(x, weight_ternary, weight_scale)
    res = run_bass_kernel_spmd(nc, in_maps, core_ids=list(range(N_CORES)),
                               trace=trace)
    full = np.concatenate([res.results[c]["out"] for c in range(N_CORES)],
                          axis=0)
    return full.reshape(B, S, DOUT).astype(np.float32), res


def kernel(x, weight_ternary, weight_scale):
    out, _ = run(x, weight_ternary, weight_scale, trace=False)
    return out


# revision 11
# speedup vs baseline: 1.8139x; 1.8139x over previous
"""BitLinear158 (LayerNorm -> int8 fake-quant -> ternary matmul -> LayerNorm)
on 8 Trainium2 NeuronCores, data-parallel over tokens.

Math notes (vs the fp32 reference):
  - Input LayerNorm's rstd cancels inside the activation quantizer:
        q = round(xn / (max|xn|/127)) = round((x-mu) * 127 / max|x-mu|)
    so the input-side sqrt/reciprocal of the variance is never needed.
  - q in [-127,127] and ternary weights {-1,0,1} are exact in bf16, and the
    PE accumulates in fp32, so the matmul integer arithmetic is exact.
  - The final LayerNorm is invariant to the per-token positive scale
    (x_scale), up to the eps term (~1e-5 relative), so x_quant*x_scale is
    never materialized; weight_scale is applied in fp32 after the matmul.
  - round-half-to-even is implemented with the fp32 magic-number trick:
    t = fma(v, c, 1.5*2^23); q = t - 1.5*2^23.
"""

from contextlib import ExitStack

import numpy as np
import ml_dtypes

N_CORES = 8
B, S, DIN, DOUT = 4, 4096, 2048, 2048
M_TOTAL = B * S
M_PER_CORE = M_TOTAL // N_CORES
P = 128
NBLK = M_PER_CORE // P          # token blocks per core
KT = DIN // P                   # contraction subtiles
NT = DOUT // 512                # psum bank tiles
EPS = 1e-5
MAGIC = float(np.float32(1.5 * 2 ** 23))

_CACHE = {}


def _build_nc(m_per_core=M_PER_CORE, repeats=1, loop_repeats=0):
    key = ("nc", m_per_core, repeats, loop_repeats)
    if key in _CACHE:
        return _CACHE[key]
    NBLK = m_per_core // P

    import concourse.bacc as bacc
    import concourse.tile as tile
    from concourse import mybir

    f32 = mybir.dt.float32
    bf16 = mybir.dt.bfloat16
    X = mybir.AxisListType.X
    Identity = mybir.ActivationFunctionType.Identity
    Sqrt = mybir.ActivationFunctionType.Sqrt

    nc = bacc.Bacc("TRN2", target_bir_lowering=False, num_devices=N_CORES,
                   name="bitlinear158")
    xs = nc.dram_tensor("xs", [m_per_core, DIN], f32, kind="ExternalInput")
    wt = nc.dram_tensor("wt", [DIN, DOUT], bf16, kind="ExternalInput")
    wsc = nc.dram_tensor("wsc", [1, DOUT], f32, kind="ExternalInput")
    out = nc.dram_tensor("out", [m_per_core, DOUT], f32, kind="ExternalOutput")

    with tile.TileContext(nc) as tc, ExitStack() as ctx:
        singles = ctx.enter_context(tc.tile_pool(name="singles", bufs=1))
        xp = ctx.enter_context(tc.tile_pool(name="xp", bufs=3))
        xcp = ctx.enter_context(tc.tile_pool(name="xcp", bufs=2))
        qp = ctx.enter_context(tc.tile_pool(name="qp", bufs=2))
        qtp = ctx.enter_context(tc.tile_pool(name="qtp", bufs=2))
        gp = ctx.enter_context(tc.tile_pool(name="gp", bufs=2))
        stp = ctx.enter_context(tc.tile_pool(name="stp", bufs=8))
        psp = ctx.enter_context(tc.tile_pool(name="psp", bufs=2, space="PSUM"))

        w_sb = singles.tile([P, KT, DOUT], bf16)
        nc.sync.dma_start(out=w_sb, in_=wt.rearrange("(kt p) n -> p kt n", p=P))
        wsc_sb = singles.tile([P, DOUT], f32)
        nc.sync.dma_start(out=wsc_sb, in_=wsc.ap().to_broadcast((P, DOUT)))
        eps_t = singles.tile([P, 1], f32)
        nc.vector.memset(eps_t, EPS)
        magic_t = singles.tile([P, 1], f32)
        nc.vector.memset(magic_t, MAGIC)

        loop_ctx = (tc.For_i(0, loop_repeats, 1) if loop_repeats
                    else ExitStack())
        with loop_ctx:
         for rep in range(repeats):
          for blk in range(NBLK):
            rows = slice(blk * P, (blk + 1) * P)

            x_t = xp.tile([P, DIN], f32)
            nc.gpsimd.dma_start(out=x_t, in_=xs[rows, :])

            # ---- input LayerNorm + 8-bit absmax quant (rstd-free form) ----
            ssum = stp.tile([P, 1], f32)
            nc.vector.reduce_sum(out=ssum, in_=x_t, axis=X)
            negmu = stp.tile([P, 1], f32)
            nc.vector.tensor_scalar_mul(negmu, ssum, -1.0 / DIN)

            xc_t = xcp.tile([P, DIN], f32)          # x - mu
            nc.scalar.activation(out=xc_t, in_=x_t, func=Identity,
                                 bias=negmu, scale=1.0)

            amax = stp.tile([P, 1], f32)            # max |x - mu|
            nc.vector.tensor_reduce(out=amax, in_=xc_t, axis=X,
                                    op=mybir.AluOpType.max,
                                    apply_absolute_value=True)
            c127 = stp.tile([P, 1], f32)            # 127 / amax
            nc.vector.reciprocal(out=c127, in_=amax)
            nc.vector.tensor_scalar_mul(c127, c127, 127.0)

            # t = xc*c + MAGIC  (rounds to integer, RNE);  q = t - MAGIC
            nc.scalar.activation(out=xc_t, in_=xc_t, func=Identity,
                                 bias=magic_t, scale=c127)
            q_t = qp.tile([P, DIN], bf16)
            nc.vector.tensor_scalar(q_t, xc_t, MAGIC, None,
                                    op0=mybir.AluOpType.subtract)

            # ---- transpose q to contraction-major for the PE ----
            # 4 blocked xbar transposes: qT3[:, kt, :] = q[:, kt*128:+128].T
            qT3 = qtp.tile([P, KT, P], bf16)
            NSPLIT = 4
            per = KT // NSPLIT
            for s in range(NSPLIT):
                nc.sync.dma_start_transpose(
                    out=qT3[:, s * per:(s + 1) * per, :],
                    in_=q_t[:, s * per * P:(s + 1) * per * P])
            qT_t = qT3.rearrange("p kt m -> p (kt m)")

            # ---- exact integer matmul in bf16: psum = q @ ternary.T ----
            ps = psp.tile([P, DOUT], f32)
            for kt in range(KT):
                for nt in range(NT):
                    ncols = slice(nt * 512, (nt + 1) * 512)
                    nc.tensor.matmul(ps[:, ncols],
                                     lhsT=qT_t[:, kt * P:(kt + 1) * P],
                                     rhs=w_sb[:, kt, ncols],
                                     start=(kt == 0), stop=(kt == KT - 1))

            # ---- g = psum * weight_scale; output LayerNorm stats ----
            g_t = gp.tile([P, DOUT], f32)
            nc.vector.tensor_mul(g_t, ps, wsc_sb)

            st2 = stp.tile([P, 4, 6], f32)
            for sg in range(4):
                nc.vector.bn_stats(out=st2[:, sg, :],
                                   in_=g_t[:, sg * 512:(sg + 1) * 512])
            mv2 = stp.tile([P, 2], f32)
            nc.vector.bn_aggr(out=mv2, in_=st2)

            rstd2 = stp.tile([P, 1], f32)
            nc.scalar.activation(out=rstd2, in_=mv2[:, 1:2], func=Sqrt,
                                 bias=eps_t, scale=1.0)
            nc.vector.reciprocal(out=rstd2, in_=rstd2)
            nb2 = stp.tile([P, 1], f32)
            nc.vector.tensor_scalar_mul(nb2, mv2[:, 0:1], -1.0)
            nc.vector.tensor_mul(nb2, nb2, rstd2)

            nc.scalar.activation(out=g_t, in_=g_t, func=Identity,
                                 bias=nb2, scale=rstd2)
            nc.scalar.dma_start(out=out[rows, :], in_=g_t)

    nc.compile()
    _CACHE[key] = nc
    return nc


def _prep_in_maps(x, weight_ternary, weight_scale):
    xs = np.ascontiguousarray(
        np.asarray(x, dtype=np.float32).reshape(M_TOTAL, DIN))
    wt = np.ascontiguousarray(
        np.asarray(weight_ternary).astype(np.float32).T.astype(
            ml_dtypes.bfloat16))
    wsc = np.ascontiguousarray(
        np.asarray(weight_scale, dtype=np.float32).reshape(1, DOUT))
    return [
        {"xs": np.ascontiguousarray(xs[c * M_PER_CORE:(c + 1) * M_PER_CORE]),
         "wt": wt, "wsc": wsc}
        for c in range(N_CORES)
    ]


def run(x, weight_ternary, weight_scale, trace=False):
    from concourse.bass_utils import run_bass_kernel_spmd
    nc = _build_nc()
    in_maps = _prep_in_maps(x, weight_ternary, weight_scale)
    res = run_bass_kernel_spmd(nc, in_maps, core_ids=list(range(N_CORES)),
                               trace=trace)
    full = np.concatenate([res.results[c]["out"] for c in range(N_CORES)],
                          axis=0)
    return full.reshape(B, S, DOUT).astype(np.float32), res


def kernel(x, weight_ternary, weight_scale):
    out, _ = run(x, weight_ternary, weight_scale, trace=False)
    return out
